# revision 32
# baseline (speedup 1.0000x reference)
"""Trainium2 Bass kernel for nn_LinearCondensed.

Computes out[b, o] = sum_k weight[o, k] * x[b, indx_seqs[o, k]] + bias[o]
with B=2048, IN_F=OUT_F=4096, FAN_IN=32.

Strategy: the gather has no fast on-chip primitive (any materialized gather
moves 32x the data of x itself), so we densify the sparse weight matrix on
the host -- W'[o, i] = sum_{k: indx_seqs[o,k]==i} weight[o, k] -- and run a
dense fp16 matmul out = x @ W'^T + bias on the PE array (fp32 PSUM
accumulation), which streams at 1 cycle/row. OUT_F is sharded 8 ways across
cores (512 columns each), x is replicated.

Schedule: all inputs ride one HWDGE ring whose FIFO order is the arrival
schedule. Phase 1 processes b-tiles 0-7 in k-stripes of 4 k-tiles: each
0.5MB weight group + 1MB x-stripe unlocks 6.9us of PE work against 4.2us of
DMA, so the PE saturates ~11us in and never stalls. x for b-tiles 0-7 is
host-pretiled into stripe-contiguous layout so stripe DMAs stay at 8KB per
descriptor (full bandwidth). A short burst of dummy matmuls on a memset
tile warms the PE's HAM clock gate (1.2 -> 2.4 GHz) before real data lands.
"""

import os
import sys
import types

import numpy as np

import concourse.bacc as bacc
import concourse.mybir as mybir
import concourse.tile as tile
from concourse.bass_utils import run_bass_kernel_spmd

B, IN_F, OUT_F, FAN_IN = 2048, 4096, 4096, 32
NCORES = 8
OSH = OUT_F // NCORES          # 512 output features per core
P = 128                        # partitions
BT = B // P                    # 16 batch tiles
KT = IN_F // P                 # 32 contraction tiles
N = OSH                        # 512 moving columns
WG = 4                         # k-tiles per weight group / stripe
NS = KT // WG                  # 8 stripes
HB = BT // 2                   # 8 b-tiles in the striped phase

f32 = mybir.dt.float32
f16 = mybir.dt.float16

_cache = {}


def _enable_ntff_hook():
    """Register the ctypes NTFF profile hook (the image's antenv lacks
    axon_hooks); lets trace=True produce a neuron-profile under axon."""
    try:
        from antenv.axon_hooks import get_axon_ntff_profile_hook  # noqa: F401
        return
    except ImportError:
        pass
    try:
        import antenv
        from trn_agent_boot.trn_boot import _ntff_profile_via_ctypes

        mod = types.ModuleType("antenv.axon_hooks")
        holder = [None]
        mod.set_axon_ntff_profile_hook = lambda h: holder.__setitem__(0, h)
        mod.get_axon_ntff_profile_hook = lambda: holder[0]
        antenv.axon_hooks = mod
        sys.modules["antenv.axon_hooks"] = mod
        mod.set_axon_ntff_profile_hook(
            _ntff_profile_via_ctypes("/opt/axon/libaxon_pjrt.so"))
        import concourse.bass_utils as bu
        bu.upload_artifacts = lambda tmpdir: str(tmpdir)
    except Exception:
        pass


def _build():
    nc = bacc.Bacc()
    # Host-pretiled layouts (all DMAs land as >=4KB contiguous lines per
    # partition):
    #   XS[s, p, t, a, c] = x[t*128 + c, (4s+a)*128 + p]   (b-tiles 0-7)
    #   XT[j, p, a, c]    = x[(j+8)*128 + c, a*128 + p]    (b-tiles 8-15)
    #   WT[a, p, n]       = W'[o0 + n, a*128 + p]
    XS = nc.declare_dram_parameter("XS", [NS, P, HB * WG * P], f16, isOutput=False)
    XT = nc.declare_dram_parameter("XT", [BT - HB, P, KT * P], f16, isOutput=False)
    WT = nc.declare_dram_parameter("WT", [KT, P, N], f16, isOutput=False)
    BIAS = nc.declare_dram_parameter("BIAS", [P, N], f32, isOutput=False)
    OUT = nc.declare_dram_parameter("OUT", [B, N], f32, isOutput=True)

    XSv = XS.ap().rearrange("s p (t a c) -> s p t a c", t=HB, a=WG)
    XTp = XT.ap().rearrange("(j d) p q -> j p d q", d=2)

    with tile.TileContext(nc) as tc:
        with (
            tc.tile_pool(name="wpool", bufs=1) as wpool,
            tc.tile_pool(name="xpool", bufs=4) as xpool,
            tc.tile_pool(name="cpool", bufs=1) as cpool,
            tc.tile_pool(name="opool", bufs=3) as opool,
            tc.tile_pool(name="psum", bufs=8, space="PSUM") as psum,
        ):
            stiles = {}

            def load_stripe(s):
                xs = xpool.tile([P, HB, WG, P], f16, tag="xs")
                nc.sync.dma_start(xs[:], XSv[s])
                stiles[s] = xs

            def load_stripe_half(s, h):
                xs = xpool.tile([P, HB // 2, WG, P], f16, tag="xsh")
                nc.sync.dma_start(xs[:], XSv[s][:, h * 4:(h + 1) * 4])
                stiles[(s, h)] = xs

            def load_xq(t):
                # single-b-tile chunk of stripe 0 (128KB): the start ramp
                # consumes these at 0.86us of PE work per 0.35us of DMA
                xs = xpool.tile([P, WG, P], f16, tag=f"xq{t % 2}")
                nc.sync.dma_start(xs[:], XSv[0][:, t])
                stiles[(0, "q", t)] = xs

            ptiles = {}

            def load_xpair(j):
                # two phase-2 b-tiles (2MB) per DMA: t = 8+2j, 9+2j
                xs = xpool.tile([P, 2, KT, P], f16, tag="xp")
                nc.sync.dma_start(
                    xs[:], XTp[j].rearrange("p d (a q) -> p d a q", a=KT))
                ptiles[8 + 2 * j] = xs[:, 0]
                ptiles[9 + 2 * j] = xs[:, 1]

            wgroups = []

            def load_w(g):
                w = wpool.tile([P, WG, N], f16, tag=f"w{g}")
                nc.sync.dma_start(
                    w[:], WT.ap().rearrange("(g j) p n -> g p j n", j=WG)[g])
                wgroups.append(w)

            # Ring FIFO order = arrival schedule.
            load_w(0)
            for t in range(HB):
                load_xq(t)
            load_w(1)
            load_stripe_half(1, 0)
            load_stripe_half(1, 1)
            for s in range(2, NS):
                load_w(s)
                load_stripe(s)
            brow = cpool.tile([P, N], f32)
            nc.sync.dma_start(brow[:], BIAS[:])
            load_xpair(0)
            load_xpair(1)
            wtiles = [wgroups[a // WG][:, a % WG, :] for a in range(KT)]

            # PE warmup on a memset tile: keeps the HAM activity window busy
            # so real matmuls start at 2.4 GHz. The warm accumulator is the
            # first allocation of the psum "acc" ring (9th alloc reuses it).
            wsrc = cpool.tile([P, N], f16, tag="wsrc")
            nc.vector.memset(wsrc[:], 0.0)
            wacc = psum.tile([P, N], f32, tag="acc")
            NWARM = 9
            for i in range(NWARM):
                nc.tensor.matmul(wacc[:], wsrc[:, :P], wsrc[:],
                                 start=(i == 0), stop=(i == NWARM - 1))

            def finish_tile(t, acc):
                osb = opool.tile([P, N], f32, tag="osb")
                nc.vector.tensor_tensor(osb[:], acc[:], brow[:], mybir.AluOpType.add)
                nc.scalar.dma_start(OUT.ap()[t * P:(t + 1) * P, :], osb[:])

            # Phase 1: b-tiles 0-7, k-stripe order.
            accs = [psum.tile([P, N], f32, name=f"acc{t}", tag="acc")
                    for t in range(HB)]

            def mm(t, a, lhsT):
                nc.tensor.matmul(
                    accs[t][:], lhsT, wtiles[a][:],
                    start=(a == 0), stop=(a == KT - 1),
                )

            for t in range(HB):
                xs = stiles[(0, "q", t)]
                for a in range(WG):
                    mm(t, a, xs[:, a, :])
            for h in range(2):
                xs = stiles[(1, h)]
                for a in range(WG, 2 * WG):
                    for t in range(4 * h, 4 * h + 4):
                        mm(t, a, xs[:, t - 4 * h, a - WG, :])
            for s in range(2, NS):
                xs = stiles[s]
                for a in range(s * WG, (s + 1) * WG):
                    for t in range(HB):
                        mm(t, a, xs[:, t, a - s * WG, :])
            for t in range(HB):
                finish_tile(t, accs[t])

            # Phase 2: b-tiles 8-15, k-inner, x streamed just in time in
            # 2-tile pairs.
            for t in range(HB, BT - 1):
                if t in (12, 14):
                    load_xpair((t - 8) // 2)
                acc = psum.tile([P, N], f32, tag="acc")
                for a in range(KT):
                    nc.tensor.matmul(
                        acc[:], ptiles[t][:, a, :], wtiles[a][:],
                        start=(a == 0), stop=(a == KT - 1),
                    )
                finish_tile(t, acc)

            # Last b-tile: split by output columns so the first half's
            # drain + store overlaps the second half's matmuls, shortening
            # the serial tail after the final matmul.
            t = BT - 1
            H = N // 2
            for h in range(2):
                # separate psum tiles per half: sharing one tile serializes
                # half-1's matmuls behind half-0's DVE drain
                acc = psum.tile([P, N], f32, tag="acc")
                cols = slice(h * H, (h + 1) * H)
                for a in range(KT):
                    nc.tensor.matmul(
                        acc[:, 0:H], ptiles[t][:, a, :], wtiles[a][:, cols],
                        start=(a == 0), stop=(a == KT - 1),
                    )
                osb = opool.tile([P, H], f32, tag="osbh")
                nc.vector.tensor_tensor(osb[:], acc[:, 0:H], brow[:, cols],
                                        mybir.AluOpType.add)
                nc.scalar.dma_start(OUT.ap()[t * P:(t + 1) * P, cols], osb[:])

    nc.compile()
    return nc


def kernel(x, weight, bias, indx_seqs):
    x = np.asarray(x, dtype=np.float32)
    weight = np.asarray(weight, dtype=np.float32)
    bias = np.asarray(bias, dtype=np.float32)
    indx_seqs = np.asarray(indx_seqs)

    if "nc" not in _cache:
        _cache["nc"] = _build()
    nc = _cache["nc"]

    # Densify sparse weights: W'[o, i] += weight[o, k] at i = indx_seqs[o, k]
    wd = np.zeros((OUT_F, IN_F), dtype=np.float32)
    np.add.at(wd, (np.arange(OUT_F)[:, None], indx_seqs), weight)

    # Host pre-tiling (fp16 operands).
    # xt_full[t, p, a, c] = x[t*128+c, a*128+p]
    xt_full = np.ascontiguousarray(
        x.reshape(BT, P, KT, P).transpose(0, 3, 2, 1).astype(np.float16))
    # XS[s, p, t, a, c] for b-tiles 0-7 (stripe-contiguous)
    xs = np.ascontiguousarray(
        xt_full[:HB].reshape(HB, P, NS, WG, P).transpose(2, 1, 0, 3, 4)
    ).reshape(NS, P, HB * WG * P)
    xt_hi = np.ascontiguousarray(xt_full[HB:]).reshape(BT - HB, P, KT * P)
    in_maps = []
    for c in range(NCORES):
        wshard = wd[c * OSH:(c + 1) * OSH]            # (512, 4096)
        # WT[a, p, n] = W'[o0+n, a*128+p]
        wt = np.ascontiguousarray(
            wshard.reshape(OSH, KT, P).transpose(1, 2, 0).astype(np.float16))
        in_maps.append({
            "XS": xs,
            "XT": xt_hi,
            "WT": wt,
            "BIAS": np.ascontiguousarray(np.broadcast_to(bias[c * OSH:(c + 1) * OSH], (P, N))),
        })

    trace = bool(int(os.environ.get("BASSK_TRACE", "0"))) or bool(
        os.environ.get("BASS_TRACE"))
    if trace:
        _enable_ntff_hook()
    res = run_bass_kernel_spmd(
        nc, in_maps, list(range(NCORES)), trace=trace,
        trace_cores=list(range(NCORES)) if trace else None,
    )
    _cache["last_results"] = res

    out = np.concatenate([res.results[c]["OUT"] for c in range(NCORES)], axis=1)
    return out


# revision 35
# speedup vs baseline: 1.1509x; 1.1509x over previous
"""Trainium2 Bass kernel for nn_LinearCondensed.

Computes out[b, o] = sum_k weight[o, k] * x[b, indx_seqs[o, k]] + bias[o]
with B=2048, IN_F=OUT_F=4096, FAN_IN=32.

Strategy: the gather has no fast on-chip primitive (any materialized gather
moves 32x the data of x itself), so we densify the sparse weight matrix on
the host -- W'[o, i] = sum_{k: indx_seqs[o,k]==i} weight[o, k] -- and run a
dense fp16 matmul out = x @ W'^T + bias on the PE array (fp32 PSUM
accumulation), which streams at 1 cycle/row. OUT_F is sharded 8 ways across
cores (512 columns each), x is replicated.

Schedule: all inputs ride one HWDGE ring whose FIFO order is the arrival
schedule. Phase 1 processes b-tiles 0-7 in k-stripes of 4 k-tiles: each
0.5MB weight group + 1MB x-stripe unlocks 6.9us of PE work against 4.2us of
DMA, so the PE saturates ~11us in and never stalls. x for b-tiles 0-7 is
host-pretiled into stripe-contiguous layout so stripe DMAs stay at 8KB per
descriptor (full bandwidth). A short burst of dummy matmuls on a memset
tile warms the PE's HAM clock gate (1.2 -> 2.4 GHz) before real data lands.
"""

import os
import sys
import types

import numpy as np

import concourse.bacc as bacc
import concourse.mybir as mybir
import concourse.tile as tile
from concourse.bass_utils import run_bass_kernel_spmd

B, IN_F, OUT_F, FAN_IN = 2048, 4096, 4096, 32
NCORES = 8
OSH = OUT_F // NCORES          # 512 output features per core
P = 128                        # partitions
BT = B // P                    # 16 batch tiles
KT = IN_F // P                 # 32 contraction tiles
N = OSH                        # 512 moving columns
WG = 4                         # k-tiles per weight group / stripe
NS = KT // WG                  # 8 stripes
HB = BT // 2                   # 8 b-tiles in the striped phase

f32 = mybir.dt.float32
f16 = mybir.dt.float16

_cache = {}


def _enable_ntff_hook():
    """Register the ctypes NTFF profile hook (the image's antenv lacks
    axon_hooks); lets trace=True produce a neuron-profile under axon."""
    try:
        from antenv.axon_hooks import get_axon_ntff_profile_hook  # noqa: F401
        return
    except ImportError:
        pass
    try:
        import antenv
        from trn_agent_boot.trn_boot import _ntff_profile_via_ctypes

        mod = types.ModuleType("antenv.axon_hooks")
        holder = [None]
        mod.set_axon_ntff_profile_hook = lambda h: holder.__setitem__(0, h)
        mod.get_axon_ntff_profile_hook = lambda: holder[0]
        antenv.axon_hooks = mod
        sys.modules["antenv.axon_hooks"] = mod
        mod.set_axon_ntff_profile_hook(
            _ntff_profile_via_ctypes("/opt/axon/libaxon_pjrt.so"))
        import concourse.bass_utils as bu
        bu.upload_artifacts = lambda tmpdir: str(tmpdir)
    except Exception:
        pass


def _build():
    nc = bacc.Bacc()
    # Host-pretiled layouts (all DMAs land as >=4KB contiguous lines per
    # partition):
    #   XS[s, p, t, a, c] = x[t*128 + c, (4s+a)*128 + p]   (b-tiles 0-7)
    #   XT[j, p, a, c]    = x[(j+8)*128 + c, a*128 + p]    (b-tiles 8-15)
    #   WT[a, p, n]       = W'[o0 + n, a*128 + p]
    XS = nc.declare_dram_parameter("XS", [NS, P, HB * WG * P], f16, isOutput=False)
    XT = nc.declare_dram_parameter("XT", [BT - HB, P, KT * P], f16, isOutput=False)
    WT = nc.declare_dram_parameter("WT", [KT, P, N], f16, isOutput=False)
    BIAS = nc.declare_dram_parameter("BIAS", [P, N], f32, isOutput=False)
    OUT = nc.declare_dram_parameter("OUT", [B, N], f32, isOutput=True)

    XSv = XS.ap().rearrange("s p (t a c) -> s p t a c", t=HB, a=WG)
    XTp = XT.ap().rearrange("(j d) p q -> j p d q", d=2)

    with tile.TileContext(nc) as tc:
        with (
            tc.tile_pool(name="wpool", bufs=1) as wpool,
            tc.tile_pool(name="xpool", bufs=4) as xpool,
            tc.tile_pool(name="cpool", bufs=1) as cpool,
            tc.tile_pool(name="opool", bufs=3) as opool,
            tc.tile_pool(name="psum", bufs=8, space="PSUM") as psum,
        ):
            stiles = {}

            def load_stripe(s):
                xs = xpool.tile([P, HB, WG, P], f16, tag="xs")
                nc.sync.dma_start(xs[:], XSv[s])
                stiles[s] = xs

            def load_stripe_half(s, h):
                xs = xpool.tile([P, HB // 2, WG, P], f16, tag="xsh")
                nc.sync.dma_start(xs[:], XSv[s][:, h * 4:(h + 1) * 4])
                stiles[(s, h)] = xs

            def load_xq(t):
                # single-b-tile chunk of stripe 0 (128KB): the start ramp
                # consumes these at 0.86us of PE work per 0.35us of DMA
                xs = xpool.tile([P, WG, P], f16, tag=f"xq{t % 2}")
                nc.sync.dma_start(xs[:], XSv[0][:, t])
                stiles[(0, "q", t)] = xs

            ptiles = {}

            def load_xpair(j):
                # two phase-2 b-tiles (2MB) per DMA: t = 8+2j, 9+2j
                xs = xpool.tile([P, 2, KT, P], f16, tag="xp")
                nc.sync.dma_start(
                    xs[:], XTp[j].rearrange("p d (a q) -> p d a q", a=KT))
                ptiles[8 + 2 * j] = xs[:, 0]
                ptiles[9 + 2 * j] = xs[:, 1]

            wgroups = []

            def load_w(g):
                w = wpool.tile([P, WG, N], f16, tag=f"w{g}")
                nc.sync.dma_start(
                    w[:], WT.ap().rearrange("(g j) p n -> g p j n", j=WG)[g])
                wgroups.append(w)

            # Ring FIFO order = arrival schedule.
            load_w(0)
            for t in range(HB):
                load_xq(t)
            load_w(1)
            load_stripe_half(1, 0)
            load_stripe_half(1, 1)
            for s in range(2, NS):
                load_w(s)
                load_stripe(s)
            brow = cpool.tile([P, N], f32)
            nc.sync.dma_start(brow[:], BIAS[:])
            load_xpair(0)
            load_xpair(1)
            wtiles = [wgroups[a // WG][:, a % WG, :] for a in range(KT)]

            # PE warmup on a memset tile: keeps the HAM activity window busy
            # so real matmuls start at 2.4 GHz. The warm accumulator is the
            # first allocation of the psum "acc" ring (9th alloc reuses it).
            wsrc = cpool.tile([P, N], f16, tag="wsrc")
            nc.vector.memset(wsrc[:], 0.0)
            wacc = psum.tile([P, N], f32, tag="acc")
            NWARM = 8
            for i in range(NWARM):
                nc.tensor.matmul(wacc[:], wsrc[:, :P], wsrc[:],
                                 start=(i == 0), stop=(i == NWARM - 1))

            def finish_tile(t, acc):
                osb = opool.tile([P, N], f32, tag="osb")
                nc.vector.tensor_tensor(osb[:], acc[:], brow[:], mybir.AluOpType.add)
                nc.scalar.dma_start(OUT.ap()[t * P:(t + 1) * P, :], osb[:])

            # Phase 1: b-tiles 0-7, k-stripe order.
            accs = [psum.tile([P, N], f32, name=f"acc{t}", tag="acc")
                    for t in range(HB)]

            def mm(t, a, lhsT):
                nc.tensor.matmul(
                    accs[t][:], lhsT, wtiles[a][:],
                    start=(a == 0), stop=(a == KT - 1),
                )

            for t in range(HB):
                xs = stiles[(0, "q", t)]
                for a in range(WG):
                    mm(t, a, xs[:, a, :])
            for h in range(2):
                xs = stiles[(1, h)]
                for a in range(WG, 2 * WG):
                    for t in range(4 * h, 4 * h + 4):
                        mm(t, a, xs[:, t - 4 * h, a - WG, :])
            for s in range(2, NS):
                xs = stiles[s]
                for a in range(s * WG, (s + 1) * WG):
                    for t in range(HB):
                        mm(t, a, xs[:, t, a - s * WG, :])
            for t in range(HB):
                finish_tile(t, accs[t])

            # Phase 2: b-tiles 8-15, k-inner, x streamed just in time in
            # 2-tile pairs.
            for t in range(HB, BT - 1):
                if t in (12, 14):
                    load_xpair((t - 8) // 2)
                acc = psum.tile([P, N], f32, tag="acc")
                for a in range(KT):
                    nc.tensor.matmul(
                        acc[:], ptiles[t][:, a, :], wtiles[a][:],
                        start=(a == 0), stop=(a == KT - 1),
                    )
                finish_tile(t, acc)

            # Last b-tile: split by output columns so the first half's
            # drain + store overlaps the second half's matmuls, shortening
            # the serial tail after the final matmul.
            t = BT - 1
            H = N // 2
            for h in range(2):
                # separate psum tiles per half: sharing one tile serializes
                # half-1's matmuls behind half-0's DVE drain (~0.8us stall)
                acc = psum.tile([P, N], f32, tag="acc")
                cols = slice(h * H, (h + 1) * H)
                for a in range(KT):
                    nc.tensor.matmul(
                        acc[:, 0:H], ptiles[t][:, a, :], wtiles[a][:, cols],
                        start=(a == 0), stop=(a == KT - 1),
                    )
                osb = opool.tile([P, H], f32, tag="osbh")
                nc.vector.tensor_tensor(osb[:], acc[:, 0:H], brow[:, cols],
                                        mybir.AluOpType.add)
                nc.scalar.dma_start(OUT.ap()[t * P:(t + 1) * P, cols], osb[:])

    nc.compile()
    return nc


def kernel(x, weight, bias, indx_seqs):
    x = np.asarray(x, dtype=np.float32)
    weight = np.asarray(weight, dtype=np.float32)
    bias = np.asarray(bias, dtype=np.float32)
    indx_seqs = np.asarray(indx_seqs)

    if "nc" not in _cache:
        _cache["nc"] = _build()
    nc = _cache["nc"]

    # Densify sparse weights: W'[o, i] += weight[o, k] at i = indx_seqs[o, k]
    wd = np.zeros((OUT_F, IN_F), dtype=np.float32)
    np.add.at(wd, (np.arange(OUT_F)[:, None], indx_seqs), weight)

    # Host pre-tiling (fp16 operands).
    # xt_full[t, p, a, c] = x[t*128+c, a*128+p]
    xt_full = np.ascontiguousarray(
        x.reshape(BT, P, KT, P).transpose(0, 3, 2, 1).astype(np.float16))
    # XS[s, p, t, a, c] for b-tiles 0-7 (stripe-contiguous)
    xs = np.ascontiguousarray(
        xt_full[:HB].reshape(HB, P, NS, WG, P).transpose(2, 1, 0, 3, 4)
    ).reshape(NS, P, HB * WG * P)
    xt_hi = np.ascontiguousarray(xt_full[HB:]).reshape(BT - HB, P, KT * P)
    in_maps = []
    for c in range(NCORES):
        wshard = wd[c * OSH:(c + 1) * OSH]            # (512, 4096)
        # WT[a, p, n] = W'[o0+n, a*128+p]
        wt = np.ascontiguousarray(
            wshard.reshape(OSH, KT, P).transpose(1, 2, 0).astype(np.float16))
        in_maps.append({
            "XS": xs,
            "XT": xt_hi,
            "WT": wt,
            "BIAS": np.ascontiguousarray(np.broadcast_to(bias[c * OSH:(c + 1) * OSH], (P, N))),
        })

    trace = bool(int(os.environ.get("BASSK_TRACE", "0"))) or bool(
        os.environ.get("BASS_TRACE"))
    if trace:
        _enable_ntff_hook()
    res = run_bass_kernel_spmd(
        nc, in_maps, list(range(NCORES)), trace=trace,
        trace_cores=list(range(NCORES)) if trace else None,
    )
    _cache["last_results"] = res

    out = np.concatenate([res.results[c]["OUT"] for c in range(NCORES)], axis=1)
    return out
